# revision 14
# baseline (speedup 1.0000x reference)
"""MultiHeadCrossAttention Trainium2 kernel (8-core data-parallel), v3.

Shapes (hardcoded): B=16, SQ=SE=1024, C_IN=C_ENC=256, DK=DV=64, H=8.
Sharding: batch across 8 cores (2 batches/core). 157.3us cost-model time
per core (v1 baseline: 182.3us, 1.16x); hardware rel err 5.54e-3.

Design (ACT-bound: 128 exp instructions x 1038ns = 132.9us is the floor;
PE busy 118us, DVE ~75us):
- Per head-slot s: ACT runs 8x exp([128,1024]) over the PSUM score tiles
  of head s while PE computes scores(s+1) (f32r khT/qhT, free-512
  matmuls), av(s-1), transposes, and background projections.
- AV flipped: stationary = exp(scores) chunk [128k, 128q], moving =
  vh_aug [128k, 65] -> [q, e] output, free size 65 (stationary loads are
  free in the cost model). Softmax denominators ride as the ones-column.
- Normalization is per-partition (r varies along q): reciprocal [128, 8]
  + 8x tensor_scalar per head; no partition broadcasts.
- o[q, e] -> oT[he, q] via PE transposes (bf16, is_transpose), batched
  per head-pair into one PSUM bank, evacuated with one [128,1024] DVE
  copy. Out-projection (bf16 wp -- mixed f32r x bf16 matmuls are
  rejected by the BIR verifier) consumes oT as in v1.
- PSUM: sp 3x2 banks (3-deep score pipeline kills the exp/scores
  lockstep), mm-pool 2x1 bank shared by projections/AV/transposes.
- Slot emission: score tiles t0/t1 of the next head go BEFORE the
  background-projection pops -- those units stall mid-rotation on the
  mm-pool evacuations and otherwise push the exp-gating tiles past the
  exp cadence (was a recurring ~400-500ns ACT gap per batch).
- qk/qx pools 4-deep so batch-1 prep never stalls on batch-0 readers.
- Warmup: m0 weight slab is a separate tiny DMA; q loads before x with
  the last x chunk split; the gating kh evacuation goes through ACT
  (Copy); warmup kslab PSUM comes from the sp pool. First exp ~7.5us.
- Tail: sqrt-table load hoisted right after the last exp (dummy Sqrt
  writes a y_all corner so DCE keeps it; Square/Prelu live in the same
  table set); last head's AV is kt-split (first half during the last
  exps); head 14's transposes pre-evacuated in slot 15; last batch's
  out-proj split g0-2 (during slots 14-15) + g3-combine at the tail
  (stt with one PSUM input); psq stats via ACT Square+accum (last chunk
  on DVE); per-ct BN finalize (DVE recip + ACT Sqrt -- Rsqrt activation
  is blocked in bass); applies 2x ACT Prelu + 2x DVE; per-chunk stores.

Hardware pitfalls (validated on trn2, rel err 5.54e-3 vs 2e-2 gate):
- matmul: out PSUM-only, operands SBUF-only; mixed f32r x bf16 rejected.
- scalar_tensor_tensor and tensor_reduce are DVE-only; Pool (gpsimd)
  does support tensor_scalar/tensor_tensor (probed) but at 0.42-0.6
  efficiency -- too slow for the apply.
- ACT activation(Copy, scale=[P,1] AP) computes in*scale (probed OK).
- Rsqrt/Reciprocal activations are blocked in bass (accuracy); use DVE
  reciprocal + ACT Sqrt.
- Only one non-PSUM... one non-scalar PSUM input per DVE instruction;
  PSUM accumulation groups must not interleave on the PE.
- The tile scheduler dead-code-eliminates ops whose output tile has no
  readers (dummy activations must write live tiles) and reorders
  within-engine streams by readiness.

Score-pipeline cadence law (measured: the B/S 2048-exp experiment hit
181us): for score tiles of C columns with beta PSUM buffers, the margin
is (beta-1)*(0.833*C+185) - 0.417*C - ~300ns(sems). The shipped 1024x3
has +1350ns; any bufs=2 scheme with C>=~700 is also positive. The one
remaining win this implies: uniform C=1536 tiles (3 banks, bufs=2 = 6
banks, fits) cut exp busy 132.9 -> 125.1us (+525ns margin/tile), BUT
8192 cols/head = 5x1536+512 leaves a remnant whose 1536-after-512
boundary deficit (~330ns x16) cancels the gain -- unless exp tiles
cross head boundaries with a 3-head pt ring (48 chunks = 16x1536
exactly, one global remnant at warmup). Requires: global-chunk-stream
score/exp emission, pt as a persistent [P, 3, 8, 1024] ring (WAR deps
via the tile framework), cross-head m_done guards. Est. -6 to -7us.
"""
import sys

sys.path.insert(0, "/opt/trn_rl_repo")

import numpy as np

import concourse.bacc as bacc
import concourse.tile as tile
from concourse import mybir
from concourse.bass_utils import run_bass_kernel_spmd

F32 = mybir.dt.float32
F32R = mybir.dt.float32r
BF16 = mybir.dt.bfloat16

B, SQ, SE = 16, 1024, 1024
C, DK, DV, H = 256, 64, 64, 8
BN_EPS = 1e-5
NEG_SLOPE = 0.01
N_CORES = 8
BL = B // N_CORES
P = 128
NT = SE // P   # 8 key tiles
NQT = SQ // P  # 8 query tiles
SCH = 2        # score sc-chunks per sp tile
SCW = SQ // SCH  # 512
NSLOTS = BL * H


def build_kernel(n_cores=N_CORES, with_collective=True):
    nc = bacc.Bacc("TRN2", target_bir_lowering=False, debug=False,
                   num_devices=n_cores)

    qt_d = nc.declare_dram_parameter("qt", [BL, 2, P, SQ], BF16, isOutput=False)
    xt_d = nc.declare_dram_parameter("xt", [BL, 2, P, SE], BF16, isOutput=False)
    wqk0_d = nc.declare_dram_parameter("wqk0", [P, 2, 2, 2, DK], BF16,
                                       isOutput=False)
    wqkr_d = nc.declare_dram_parameter("wqkr", [P, 2, 2, H - 2, DK], BF16,
                                       isOutput=False)
    wv_d = nc.declare_dram_parameter("wv", [P, 2, H * DV], BF16, isOutput=False)
    wp_d = nc.declare_dram_parameter("wp", [P, H // 2, C], BF16, isOutput=False)
    gb_d = nc.declare_dram_parameter("gb", [P, 2, 2], F32, isOutput=False)
    id_d = nc.declare_dram_parameter("id128", [P, P], BF16, isOutput=False)
    y_d = nc.declare_dram_parameter("y", [BL, 2, P, SQ], BF16, isOutput=True)

    with tile.TileContext(nc) as tc:
        with (
            tc.tile_pool(name="const", bufs=1) as const,
            tc.tile_pool(name="qx", bufs=4) as qxp,       # qT/xT inputs
            tc.tile_pool(name="qk", bufs=4) as qkp,       # qhT/khT projections
            tc.tile_pool(name="vh", bufs=2) as vhp,       # vh_aug values
            tc.tile_pool(name="pt", bufs=2) as ptp,       # exp(scores)
            tc.tile_pool(name="o2", bufs=3) as o2p,       # normalized [q, e]
            tc.tile_pool(name="ot", bufs=2) as otp,       # oT [he, q]
            tc.tile_pool(name="pp", bufs=1) as ppp,       # projected p (both b)
            tc.tile_pool(name="sm", bufs=3) as sm,        # small scratch
            tc.tile_pool(name="yy", bufs=1) as yyp,       # y staging
            tc.tile_pool(name="fin", bufs=1) as fin,
            tc.tile_pool(name="sp_ps", bufs=3, space="PSUM") as sp_ps,  # 3x2bk
            tc.tile_pool(name="mm_ps", bufs=2, space="PSUM") as mm_ps,  # 2x1bk
            tc.tile_pool(name="dram", bufs=1, space="DRAM") as dram,
        ):
            wqk0_sb = const.tile([P, 2, 2, 2, DK], BF16, tag="wqk0")
            wqkr_sb = const.tile([P, 2, 2, H - 2, DK], BF16, tag="wqkr")

            def wslab(qk, m):
                """[P, 2(k), 2(heads), DK] weight slab for head-pair m."""
                if m == 0:
                    return wqk0_sb[:, qk]
                return wqkr_sb[:, qk, :, 2 * (m - 1):2 * m, :]
            wv_sb = const.tile([P, 2, H * DV], BF16, tag="wv")
            wp_sb = const.tile([P, H // 2, C], BF16, tag="wp")
            gb_sb = const.tile([P, 2, 2], F32, tag="gb")
            ident = const.tile([P, P], BF16, tag="ident")

            # ---------------- helpers ----------------
            p_sb = ppp.tile([P, 2, BL, SQ], BF16, tag="p")
            psq_scratch = sm.tile([P, SCW], BF16, tag="psq")
            sq_parts = fin.tile([P, 4 * BL], F32, tag="sqp")
            s_parts = fin.tile([P, 4 * BL], F32, tag="sp_")

            preps = {}
            vh_done = {}
            m_done = {}
            pts = {}
            o2s = {}
            avs = {}
            oTs = {}

            def proj_slab(wm, src, dst, m, sc0=0, sc1=SCH, act_evac=(),
                          pool=None):
                for sc in range(sc0, sc1):
                    pj = (pool or mm_ps).tile([P, SCW], F32,
                                              tag="sp" if pool else "mm")
                    for k in range(2):
                        nc.tensor.matmul(
                            pj[:],
                            wm[:, k, :, :],
                            src[:, k, sc * SCW:(sc + 1) * SCW],
                            start=(k == 0), stop=(k == 1))
                    dst_sl = dst[:, m, sc * SCW:(sc + 1) * SCW]
                    if sc in act_evac:
                        nc.scalar.activation(
                            out=dst_sl, in_=pj[:],
                            func=mybir.ActivationFunctionType.Copy)
                    else:
                        nc.vector.tensor_copy(dst_sl, pj[:])

            def prep_start(b):
                """Load qT/xT; project head-pair 0 (unblocks scores of h0)."""
                qT = qxp.tile([P, 2, SQ], BF16, tag="qx")
                xT = qxp.tile([P, 2, SE], BF16, tag="qx")
                if b == 0:
                    # q first (its projection runs while x still streams);
                    # split the last x chunk so the kh projection can start
                    # on the first half of the keys sooner
                    nc.sync.dma_start(out=qT[:, 0, :], in_=qt_d[b, 0])
                    nc.sync.dma_start(out=qT[:, 1, :], in_=qt_d[b, 1])
                    nc.sync.dma_start(out=xT[:, 0, :], in_=xt_d[b, 0])
                    nc.sync.dma_start(out=xT[:, 1, 0:SCW],
                                      in_=xt_d[b, 1][:, 0:SCW])
                    nc.sync.dma_start(out=xT[:, 1, SCW:SE],
                                      in_=xt_d[b, 1][:, SCW:SE])
                else:
                    for k in range(2):
                        nc.sync.dma_start(out=qT[:, k, :], in_=qt_d[b, k])
                    for k in range(2):
                        nc.sync.dma_start(out=xT[:, k, :], in_=xt_d[b, k])
                qhT = qkp.tile([P, H // 2, SQ], F32R, tag="qk")
                khT = qkp.tile([P, H // 2, SE], F32R, tag="qk")
                preps[b] = (qT, xT, qhT, khT, None)
                m_done[b] = -1
                # warmup: spread the four gating evacuations over DVE + ACT
                # and keep the k-slab PSUM out of the mm pool so it does not
                # wait on the q evacuations
                proj_slab(wslab(0, 0), qT, qhT, 0)
                proj_slab(wslab(1, 0), xT, khT, 0,
                          act_evac=(0,) if b == 0 else (),
                          pool=sp_ps if b == 0 else None)
                m_done[b] = 0

            def vh_alloc(b):
                qT, xT, qhT, khT, _ = preps[b]
                vh_aug = vhp.tile([P, NT, H, DV + 1], BF16, tag="vh")
                nc.vector.memset(vh_aug[:, :, :, DV:DV + 1], 1.0)
                preps[b] = (qT, xT, qhT, khT, vh_aug)

            def vproj(b, t0, t1):
                qT, xT, qhT, khT, vh_aug = preps[b]
                for t in range(t0, t1):
                    pj = mm_ps.tile([P, H * DV], F32, tag="mm")
                    for k in range(2):
                        nc.tensor.matmul(
                            pj[:], xT[:, k, t * P:(t + 1) * P], wv_sb[:, k, :],
                            start=(k == 0), stop=(k == 1))
                    nc.vector.tensor_copy(
                        vh_aug[:, t, :, 0:DV],
                        pj.rearrange("p (h e) -> p h e", h=H))

            def prep_units(b, with_start):
                units = []
                def qkslab(m):
                    proj_slab(wslab(1, m), preps[b][1], preps[b][3], m)
                    proj_slab(wslab(0, m), preps[b][0], preps[b][2], m)
                    m_done[b] = m

                if with_start:
                    units.append(lambda: prep_start(b))
                    units.append(lambda: (vh_alloc(b), vproj(b, 0, 4)))
                    units.append(lambda: (vproj(b, 4, 8),
                                          vh_done.__setitem__(b, True)))
                for m in range(1, H // 2):
                    units.append(lambda m=m: qkslab(m))
                return units

            pre_q = []
            post_q = []
            avAs = []

            def pop_units(q, n):
                for _ in range(min(n, len(q))):
                    q.pop(0)()

            def score_tiles(s, t0, t1, out):
                """Score tiles for head-slot s into sp tiles (PSUM)."""
                b, h = divmod(s, H)
                j, par = h // 2, 64 * (h % 2)
                while m_done.get(b, -1) < j and pre_q:
                    pop_units(pre_q, 1)
                qhT, khT = preps[b][2], preps[b][3]
                for t in range(t0, t1):
                    spt = sp_ps.tile([P, SCH, SCW], F32, tag="sp")
                    for sc in range(SCH):
                        nc.tensor.matmul(
                            spt[:, sc, :],
                            khT[par:par + 64, j, t * P:(t + 1) * P],
                            qhT[par:par + 64, j, sc * SCW:(sc + 1) * SCW],
                            start=True, stop=True)
                    out.append(spt)
                return out

            def emit_exp(s, sp_list):
                pt = ptp.tile([P, NT, SQ], BF16, tag="pt")
                pts[s] = pt
                for t in range(NT):
                    nc.scalar.activation(
                        out=pt[:, t, :],
                        in_=sp_list[t].rearrange("p a b -> p (a b)"),
                        func=mybir.ActivationFunctionType.Exp,
                        scale=1.0 / np.sqrt(DK).item())

            def av_and_norm(s):
                """AV ([q, e] layout, pt stationary) + per-partition norm."""
                b, h = divmod(s, H)
                if b > 0:
                    while b not in vh_done and pre_q:
                        pop_units(pre_q, 1)
                vh_aug = preps[b][4]
                pt = pts.pop(s)
                # normalization: r varies along partitions (q)
                r = sm.tile([P, NQT], F32, tag="r")
                o2 = o2p.tile([P, NQT, DV], BF16, tag="o2")
                o2s[s] = o2
                for half in range(2):
                    avt = mm_ps.tile([P, 4, DV + 1], F32, tag="mm")
                    for qi in range(4):
                        qt = half * 4 + qi
                        for kt in range(NT):
                            nc.tensor.matmul(
                                avt[:, qi, :],
                                pt[:, kt, qt * P:(qt + 1) * P],
                                vh_aug[:, kt, h, :],
                                start=(kt == 0), stop=(kt == NT - 1))
                    nc.vector.reciprocal(
                        r[:, half * 4:(half + 1) * 4], avt[:, :, DV])
                    for qi in range(4):
                        qt = half * 4 + qi
                        nc.vector.tensor_scalar(
                            o2[:, qt, :], avt[:, qi, 0:DV],
                            r[:, qt:qt + 1], None, mybir.AluOpType.mult)

            def transp_pair(s_even, split_evac=False):
                """Transpose heads (s_even, s_even+1) into oT[:, j, :]."""
                b, h = divmod(s_even, H)
                j = h // 2
                if j == 0:
                    oTs[b] = otp.tile([P, H // 2, SQ], BF16, tag="ot",
                                      name=f"oT{b}")
                oT = oTs[b]
                tp = mm_ps.tile([P, NQT, P], BF16, tag="mm")
                o2a = o2s.pop(s_even)
                o2b = o2s.pop(s_even + 1)
                tpf = tp.rearrange("p a b -> p (a b)")
                for half in range(2):
                    for qt in range(4 * half, 4 * half + 4):
                        nc.tensor.transpose(tp[0:64, qt, :], o2a[:, qt, :],
                                            ident)
                        nc.tensor.transpose(tp[64:P, qt, :], o2b[:, qt, :],
                                            ident)
                    if split_evac:
                        nc.vector.tensor_copy(
                            oT[:, j, half * SCW:(half + 1) * SCW],
                            tpf[:, half * SCW:(half + 1) * SCW])
                if not split_evac:
                    nc.vector.tensor_copy(oT[:, j, :], tpf)

            def out_proj_sc(b, ct, sc, g0=0, g1=H // 2, psq_eng=None):
                """p[c, s] for head-pair groups [g0, g1) of one (ct, sc)
                chunk; finishes BN partial stats when g1 == H//2."""
                oT = oTs[b]
                pj = mm_ps.tile([P, SCW], F32, tag="mm")
                for g in range(g0, g1):
                    nc.tensor.matmul(
                        pj[:],
                        wp_sb[:, g, ct * P:(ct + 1) * P],
                        oT[:, g, sc * SCW:(sc + 1) * SCW],
                        start=(g == g0), stop=(g == g1 - 1))
                col = 4 * ct + 2 * b + sc
                psl = p_sb[:, ct, b, sc * SCW:(sc + 1) * SCW]
                if g1 < H // 2:
                    # partial: stash in p_sb, no stats yet
                    nc.vector.tensor_scalar(
                        psl, pj[:], 1.0, 0.0,
                        mybir.AluOpType.mult, mybir.AluOpType.add)
                    return
                if g0 > 0:
                    # combine with the stashed partial + finish stats
                    nc.vector.scalar_tensor_tensor(
                        psl, pj[:], 1.0, psl,
                        mybir.AluOpType.mult, mybir.AluOpType.add,
                        accum_out=s_parts[:, col:col + 1])
                else:
                    nc.vector.tensor_scalar(
                        psl, pj[:],
                        1.0, 0.0, mybir.AluOpType.mult, mybir.AluOpType.add,
                        accum_out=s_parts[:, col:col + 1])
                if psq_eng == "act":
                    # tail: ACT is idle there and Square is in every table
                    nc.scalar.activation(
                        out=psq_scratch[:], in_=psl,
                        func=mybir.ActivationFunctionType.Square,
                        accum_out=sq_parts[:, col:col + 1])
                else:
                    nc.vector.scalar_tensor_tensor(
                        psq_scratch[:], psl, 1.0, psl,
                        mybir.AluOpType.mult, mybir.AluOpType.mult,
                        accum_out=sq_parts[:, col:col + 1])

            # ---------------- emission ----------------
            # input DMAs + first projections
            nc.sync.dma_start(out=wqk0_sb, in_=wqk0_d[:])

            # PE p-state ramp during the DMA wait
            warm = const.tile([64, SCW], BF16, tag="warm")
            nc.vector.memset(warm, 0.0)
            for i in range(4):
                wt = sp_ps.tile([P, SCH, SCW], F32, tag="sp")
                nc.tensor.matmul(wt[0:64, 0, :], warm[:, 0:64], warm[:],
                                 start=True, stop=True)

            prep_start(0)
            nc.sync.dma_start(out=wqkr_sb, in_=wqkr_d[:])
            nc.sync.dma_start(out=wv_sb, in_=wv_d[:])
            nc.sync.dma_start(out=ident, in_=id_d[:])
            nc.sync.dma_start(out=wp_sb, in_=wp_d[:])
            nc.sync.dma_start(out=gb_sb, in_=gb_d[:])
            sp_cur = score_tiles(0, 0, NT, [])
            vh_alloc(0)
            vproj(0, 0, 8)
            vh_done[0] = True
            pre_q.extend(prep_units(0, with_start=False))
            pre_q.extend(prep_units(1, with_start=True))

            for s in range(NSLOTS):
                emit_exp(s, sp_cur)
                if s >= 1:
                    av_and_norm(s - 1)
                if s == NSLOTS - 1:
                    # head 14's rows of oT j3 transpose+evacuate during the
                    # last head's exps -- only head 15 remains on the tail
                    o2a14 = o2s[s - 1]
                    oTL = oTs[BL - 1]
                    tpa = mm_ps.tile([P, NQT, P], BF16, tag="mm", name="tpa")
                    for qt in range(NQT):
                        nc.tensor.transpose(tpa[0:64, qt, :],
                                            o2a14[:, qt, :], ident)
                    nc.vector.tensor_copy(
                        oTL[0:64, H // 2 - 1, :],
                        tpa[0:64].rearrange("p a b -> p (a b)"))
                    # last head's AV over the first half of the keys runs
                    # as soon as those exps land; evacuated to SBUF so the
                    # tail only accumulates keys 512:1024 and combines
                    ptL15 = pts[s]
                    vhL = preps[BL - 1][4]
                    for half in range(2):
                        avt = mm_ps.tile([P, 4, DV + 1], F32, tag="mm",
                                         name=f"avA{half}")
                        for qi in range(4):
                            qt = half * 4 + qi
                            for kt in range(NT // 2):
                                nc.tensor.matmul(
                                    avt[:, qi, :],
                                    ptL15[:, kt, qt * P:(qt + 1) * P],
                                    vhL[:, kt, H - 1, :],
                                    start=(kt == 0), stop=(kt == NT // 2 - 1))
                        avA = sm.tile([P, 4, DV + 1], F32, tag="avp",
                                      name=f"avAs{half}")
                        avAs.append(avA)
                        nc.vector.tensor_copy(avA, avt)
                if s >= 2 and s % 2 == 0:
                    transp_pair(s - 2)
                    if (s - 2) % H == 6:  # j3 done -> queue b's out_proj
                        bb = (s - 2) // H
                        for sc in range(SCH):
                            for ct in range(2):
                                post_q.append(
                                    lambda ct=ct, sc=sc, bb=bb: out_proj_sc(
                                        bb, ct, sc))
                    if s == NSLOTS - 2:
                        # partial out-proj (head-pairs g0-g2) for the last
                        # batch -- oT j0..j2 are evacuated by now
                        for sc in range(SCH):
                            for ct in range(2):
                                post_q.append(
                                    lambda ct=ct, sc=sc: out_proj_sc(
                                        BL - 1, ct, sc, g0=0, g1=3))
                if s + 1 < NSLOTS:
                    sp_cur = score_tiles(s + 1, 0, 2, [])
                pop_units(pre_q, 2 if s < 2 else 1)
                if s + 1 < NSLOTS:
                    score_tiles(s + 1, 2, 4, sp_cur)
                pop_units(post_q, 2 if s >= NSLOTS - 2 else 1)
                if s + 1 < NSLOTS:
                    score_tiles(s + 1, 4, NT, sp_cur)

            # ---- BN finalize/apply helpers ----
            n_total = float(B * SQ) if with_collective else float(BL * SQ)
            stats = fin.tile([P, 2, 2], F32, tag="stats")  # [c, ct, {s, s2}]
            a_ap = fin.tile([P, 2], F32, tag="a")
            b_ap = fin.tile([P, 2], F32, tag="b")
            mean2 = fin.tile([P, 2], F32, tag="mean2")
            msq2 = fin.tile([P, 2], F32, tag="msq2")
            var2 = fin.tile([P, 2], F32, tag="var2")
            iv2 = fin.tile([P, 2], F32, tag="iv2")
            rstd2 = fin.tile([P, 2], F32, tag="rstd2")
            bm2 = fin.tile([P, 2], F32, tag="bm2")
            y_all = yyp.tile([P, 2, BL, SQ], BF16, tag="yall")

            def stats_ct(ct):
                nc.vector.tensor_reduce(
                    stats[:, ct:ct + 1, 0],
                    s_parts[:, 4 * ct:4 * ct + 4].rearrange(
                        "p (c x) -> p c x", c=1),
                    mybir.AxisListType.X, mybir.AluOpType.add)
                nc.vector.tensor_reduce(
                    stats[:, ct:ct + 1, 1],
                    sq_parts[:, 4 * ct:4 * ct + 4].rearrange(
                        "p (c x) -> p c x", c=1),
                    mybir.AxisListType.X, mybir.AluOpType.add)

            def finalize_ct(ct, g_sb):
                c = slice(ct, ct + 1)
                nc.vector.tensor_scalar(mean2[:, c], g_sb[:, c, 0],
                                        1.0 / n_total, None,
                                        mybir.AluOpType.mult)
                nc.vector.tensor_scalar(msq2[:, c], g_sb[:, c, 1],
                                        1.0 / n_total, None,
                                        mybir.AluOpType.mult)
                nc.vector.tensor_mul(var2[:, c], mean2[:, c], mean2[:, c])
                nc.vector.tensor_sub(var2[:, c], msq2[:, c], var2[:, c])
                nc.vector.tensor_scalar(var2[:, c], var2[:, c], 1.0, BN_EPS,
                                        mybir.AluOpType.mult,
                                        mybir.AluOpType.add)
                nc.vector.reciprocal(iv2[:, c], var2[:, c])
                # rstd = sqrt(1/(var+eps)); Sqrt set also holds Prelu
                nc.scalar.activation(out=rstd2[:, c], in_=iv2[:, c],
                                     func=mybir.ActivationFunctionType.Sqrt)
                nc.vector.tensor_mul(a_ap[:, c], rstd2[:, c], gb_sb[:, c, 0])
                nc.vector.tensor_mul(bm2[:, c], mean2[:, c], a_ap[:, c])
                nc.vector.tensor_sub(b_ap[:, c], gb_sb[:, c, 1], bm2[:, c])

            def apply_act(b, ct):
                nc.scalar.activation(
                    out=y_all[:, ct, b, :], in_=p_sb[:, ct, b, :],
                    func=mybir.ActivationFunctionType.Prelu,
                    scale=a_ap[:, ct:ct + 1], bias=b_ap[:, ct:ct + 1],
                    alpha=NEG_SLOPE)

            def apply_vec(b, ct):
                yt = sm.tile([P, SQ], BF16, tag="yt")
                nc.vector.tensor_scalar(
                    yt, p_sb[:, ct, b, :], a_ap[:, ct:ct + 1],
                    b_ap[:, ct:ct + 1],
                    mybir.AluOpType.mult, mybir.AluOpType.add)
                nc.vector.scalar_tensor_tensor(
                    y_all[:, ct, b, :], yt, NEG_SLOPE, yt,
                    mybir.AluOpType.mult, mybir.AluOpType.max)

            def finish_ct(ct):
                finalize_ct(ct, stats)
                apply_act(0, ct)
                if ct == 0:
                    apply_act(1, ct)
                else:
                    apply_vec(1, ct)
                nc.sync.dma_start(out=y_d[0, ct], in_=y_all[:, ct, 0, :])
                nc.sync.dma_start(out=y_d[1, ct], in_=y_all[:, ct, 1, :])

            # ---------------- attention tail ----------------
            # last head: av / norm / transpose / evacuate / g3 out-proj,
            # pipelined per qt-half so every engine starts early
            pop_units(post_q, len(post_q))
            # hoist the sqrt-table load off the critical path: everything
            # ACT does from here on (Square/Sqrt/Prelu) lives in the
            # sqrt_and_others set, so switch tables now while ACT is idle.
            # The dummy writes into y_all (which has real readers) so it
            # survives dead-code elimination; the apply overwrites it.
            nc.scalar.activation(out=y_all[0:1, 0, 0, 0:1],
                                 in_=ident[0:1, 0:1],
                                 func=mybir.ActivationFunctionType.Sqrt)
            sL = NSLOTS - 1
            bL = BL - 1
            vh_aug = preps[bL][4]
            ptL = pts.pop(sL)
            o2a = o2s.pop(sL - 1)
            o2b = o2p.tile([P, NQT, DV], BF16, tag="o2", name="o2last")
            rL = sm.tile([P, NQT], F32, tag="r", name="rlast")
            oT = oTs[bL]
            for half in range(2):
                avt = mm_ps.tile([P, 4, DV + 1], F32, tag="mm",
                                 name=f"avl{half}")
                for qi in range(4):
                    qt = half * 4 + qi
                    for kt in range(NT // 2, NT):
                        nc.tensor.matmul(
                            avt[:, qi, :],
                            ptL[:, kt, qt * P:(qt + 1) * P],
                            vh_aug[:, kt, H - 1, :],
                            start=(kt == NT // 2), stop=(kt == NT - 1))
                cmb = sm.tile([P, 4, DV + 1], F32, tag="avp",
                              name=f"cmb{half}")
                nc.vector.scalar_tensor_tensor(
                    cmb, avt[:], 1.0, avAs[half],
                    mybir.AluOpType.mult, mybir.AluOpType.add)
                nc.vector.reciprocal(
                    rL[:, half * 4:(half + 1) * 4], cmb[:, :, DV])
                for qi in range(4):
                    qt = half * 4 + qi
                    if qi < 2:
                        nc.vector.tensor_scalar(
                            o2b[:, qt, :], cmb[:, qi, 0:DV],
                            rL[:, qt:qt + 1], None, mybir.AluOpType.mult)
                    else:
                        nc.scalar.activation(
                            out=o2b[:, qt, :], in_=cmb[:, qi, 0:DV],
                            func=mybir.ActivationFunctionType.Copy,
                            scale=rL[:, qt:qt + 1])
            for half in range(2):
                tph = mm_ps.tile([P, 4, P], BF16, tag="mm",
                                 name=f"tpl{half}")
                for qi in range(4):
                    qt = half * 4 + qi
                    nc.tensor.transpose(tph[64:P, qi, :], o2b[:, qt, :],
                                        ident)
                nc.vector.tensor_copy(
                    oT[64:P, H // 2 - 1, half * SCW:(half + 1) * SCW],
                    tph[64:P].rearrange("p a b -> p (a b)"))
            # final chunks ct-major; finalize both cts before the applies
            out_proj_sc(bL, 0, 0, g0=3, psq_eng="act")
            out_proj_sc(bL, 0, 1, g0=3, psq_eng="act")
            stats_ct(0)
            out_proj_sc(bL, 1, 0, g0=3, psq_eng="act")
            out_proj_sc(bL, 1, 1, g0=3)
            stats_ct(1)
            if not with_collective:
                finalize_ct(0, stats)
                finalize_ct(1, stats)
                apply_act(0, 0)
                apply_vec(1, 0)
                nc.sync.dma_start(out=y_d[0, 0], in_=y_all[:, 0, 0, :])
                nc.sync.dma_start(out=y_d[1, 0], in_=y_all[:, 0, 1, :])
                apply_act(0, 1)
                apply_vec(1, 1)
                nc.sync.dma_start(out=y_d[0, 1], in_=y_all[:, 1, 0, :])
                nc.sync.dma_start(out=y_d[1, 1], in_=y_all[:, 1, 1, :])

            # ---- collective path: all-reduce stats, then finalize ----
            if with_collective:
                ar_in = dram.tile([P, 4], F32)
                ar_out = dram.tile([P, 4], F32)
                nc.sync.dma_start(out=ar_in[:],
                                  in_=stats.rearrange("p a b -> p (a b)"))
                nc.gpsimd.collective_compute(
                    "AllReduce", mybir.AluOpType.add,
                    replica_groups=[list(range(n_cores))],
                    ins=[ar_in.opt()], outs=[ar_out.opt()])
                g_sb = fin.tile([P, 2, 2], F32, tag="g")
                nc.sync.dma_start(out=g_sb.rearrange("p a b -> p (a b)"),
                                  in_=ar_out[:])
                for ct in range(2):
                    finalize_ct(ct, g_sb)
                    apply_act(0, ct)
                    apply_vec(1, ct)
                    nc.sync.dma_start(out=y_d[0, ct],
                                      in_=y_all[:, ct, 0, :])
                    nc.sync.dma_start(out=y_d[1, ct],
                                      in_=y_all[:, ct, 1, :])

    nc.compile()
    return nc


def prep_weights(Wq, Wk, Wv, Wp, gamma, beta):
    import ml_dtypes
    wq = np.ascontiguousarray(
        Wq.transpose(2, 0, 1).reshape(2, P, H, DK)
        .transpose(1, 0, 2, 3)).astype(ml_dtypes.bfloat16)
    wk = np.ascontiguousarray(
        Wk.transpose(2, 0, 1).reshape(2, P, H, DK)
        .transpose(1, 0, 2, 3)).astype(ml_dtypes.bfloat16)
    wqk = np.stack([wq, wk], axis=1)  # [P, 2(qk), 2(kc), H, DK]
    wqk0 = np.ascontiguousarray(wqk[:, :, :, 0:2, :])
    wqkr = np.ascontiguousarray(wqk[:, :, :, 2:, :])
    wv = np.ascontiguousarray(
        Wv.transpose(2, 0, 1).reshape(2, P, H * DV)
        .transpose(1, 0, 2)).astype(ml_dtypes.bfloat16)
    # wp: [128 (he within group), group, c] with he = h*64+e head-major
    wpT = Wp.T.reshape(H // 2, P, C)  # [g, he%128, c]
    wp = np.ascontiguousarray(wpT.transpose(1, 0, 2)).astype(ml_dtypes.bfloat16)
    # gamma/beta in [c%128, ct, {gamma,beta}]
    gb = np.stack([gamma.reshape(2, P), beta.reshape(2, P)], axis=-1)
    gb = np.ascontiguousarray(gb.transpose(1, 0, 2)).astype(np.float32)
    ident = np.eye(P, dtype=ml_dtypes.bfloat16)
    return (wqk0, wqkr), wv, wp, gb, ident


_NC_CACHE = {}


def kernel(x, q, Wq, Wk, Wv, Wp, gamma, beta):
    x = np.asarray(x, dtype=np.float32)
    q = np.asarray(q, dtype=np.float32)
    (wqk0, wqkr), wv, wp, gb, ident = prep_weights(
        np.asarray(Wq, np.float32), np.asarray(Wk, np.float32),
        np.asarray(Wv, np.float32), np.asarray(Wp, np.float32),
        np.asarray(gamma, np.float32), np.asarray(beta, np.float32))

    if "nc" not in _NC_CACHE:
        _NC_CACHE["nc"] = build_kernel()
    nc = _NC_CACHE["nc"]

    import ml_dtypes

    # host-side transpose: [BL, S, C] -> [BL, 2, 128, S] (bf16)
    def t_in(a):
        return np.ascontiguousarray(
            a.transpose(0, 2, 1).reshape(a.shape[0], 2, P, a.shape[1])
        ).astype(ml_dtypes.bfloat16)

    in_maps = []
    for i in range(N_CORES):
        in_maps.append({
            "qt": t_in(q[i * BL:(i + 1) * BL]),
            "xt": t_in(x[i * BL:(i + 1) * BL]),
            "wqk0": wqk0, "wqkr": wqkr, "wv": wv, "wp": wp, "gb": gb,
            "id128": ident,
        })
    res = run_bass_kernel_spmd(nc, in_maps, list(range(N_CORES)))
    outs = []
    for i in range(N_CORES):
        y = np.asarray(res.results[i]["y"]).astype(np.float32)
        y = y.reshape(BL, 2, P, SQ).transpose(0, 3, 1, 2).reshape(BL, SQ, C)
        outs.append(y)
    return np.concatenate(outs, axis=0)


# revision 15
# speedup vs baseline: 1.0042x; 1.0042x over previous
"""MultiHeadCrossAttention Trainium2 kernel (8-core data-parallel), v3.

Shapes (hardcoded): B=16, SQ=SE=1024, C_IN=C_ENC=256, DK=DV=64, H=8.
Sharding: batch across 8 cores (2 batches/core). 157.3us cost-model time
per core (v1 baseline: 182.3us, 1.16x); hardware rel err 5.54e-3.

Design (ACT-bound: 128 exp instructions x 1038ns = 132.9us is the floor;
PE busy 118us, DVE ~75us):
- Per head-slot s: ACT runs 8x exp([128,1024]) over the PSUM score tiles
  of head s while PE computes scores(s+1) (f32r khT/qhT, free-512
  matmuls), av(s-1), transposes, and background projections.
- AV flipped: stationary = exp(scores) chunk [128k, 128q], moving =
  vh_aug [128k, 65] -> [q, e] output, free size 65 (stationary loads are
  free in the cost model). Softmax denominators ride as the ones-column.
- Normalization is per-partition (r varies along q): reciprocal [128, 8]
  + 8x tensor_scalar per head; no partition broadcasts.
- o[q, e] -> oT[he, q] via PE transposes (bf16, is_transpose), batched
  per head-pair into one PSUM bank, evacuated with one [128,1024] DVE
  copy. Out-projection (bf16 wp -- mixed f32r x bf16 matmuls are
  rejected by the BIR verifier) consumes oT as in v1.
- PSUM: sp 3x2 banks (3-deep score pipeline kills the exp/scores
  lockstep), mm-pool 2x1 bank shared by projections/AV/transposes.
- Slot emission: score tiles t0/t1 of the next head go BEFORE the
  background-projection pops -- those units stall mid-rotation on the
  mm-pool evacuations and otherwise push the exp-gating tiles past the
  exp cadence (was a recurring ~400-500ns ACT gap per batch).
- qk/qx pools 4-deep so batch-1 prep never stalls on batch-0 readers.
- Warmup: m0 weight slab is a separate tiny DMA; q loads before x with
  the last x chunk split; the gating kh evacuation goes through ACT
  (Copy); warmup kslab PSUM comes from the sp pool. First exp ~7.5us.
- Tail: sqrt-table load hoisted right after the last exp (dummy Sqrt
  writes a y_all corner so DCE keeps it; Square/Prelu live in the same
  table set); last head's AV is kt-split (first half during the last
  exps); head 14's transposes pre-evacuated in slot 15; last batch's
  out-proj split g0-2 (during slots 14-15) + g3-combine at the tail
  (stt with one PSUM input); psq stats via ACT Square+accum (last chunk
  on DVE); per-ct BN finalize (DVE recip + ACT Sqrt -- Rsqrt activation
  is blocked in bass); applies 2x ACT Prelu + 2x DVE; per-chunk stores.

Hardware pitfalls (validated on trn2, rel err 5.54e-3 vs 2e-2 gate):
- matmul: out PSUM-only, operands SBUF-only; mixed f32r x bf16 rejected.
- scalar_tensor_tensor and tensor_reduce are DVE-only; Pool (gpsimd)
  does support tensor_scalar/tensor_tensor (probed) but at 0.42-0.6
  efficiency -- too slow for the apply.
- ACT activation(Copy, scale=[P,1] AP) computes in*scale (probed OK).
- Rsqrt/Reciprocal activations are blocked in bass (accuracy); use DVE
  reciprocal + ACT Sqrt.
- Only one non-PSUM... one non-scalar PSUM input per DVE instruction;
  PSUM accumulation groups must not interleave on the PE.
- The tile scheduler dead-code-eliminates ops whose output tile has no
  readers (dummy activations must write live tiles) and reorders
  within-engine streams by readiness.

Score-pipeline cadence law (measured: the B/S 2048-exp experiment hit
181us): for score tiles of C columns with beta PSUM buffers, the margin
is (beta-1)*(0.833*C+185) - 0.417*C - ~300ns(sems). The shipped 1024x3
has +1350ns; any bufs=2 scheme with C>=~700 is also positive. The one
remaining win this implies: uniform C=1536 tiles (3 banks, bufs=2 = 6
banks, fits) cut exp busy 132.9 -> 125.1us (+525ns margin/tile), BUT
8192 cols/head = 5x1536+512 leaves a remnant whose 1536-after-512
boundary deficit (~330ns x16) cancels the gain -- unless exp tiles
cross head boundaries with a 3-head pt ring (48 chunks = 16x1536
exactly, one global remnant at warmup). Requires: global-chunk-stream
score/exp emission, pt as a persistent [P, 3, 8, 1024] ring (WAR deps
via the tile framework), cross-head m_done guards. Est. -6 to -7us.
MEASURED closure of the within-head variants: 5x1536+512 with the
remnant in the mm pool = 171.2us (the remnant's late exp poisons the
av rotation); remnant in the big pool = 162.0us (boundary deficit);
4x1536+2x1024 (every window >= +98ns) = 157.4us -- the 5.9us of exp
savings is consumed almost exactly by bufs=2 queueing slip
(~400-500ns/tile effective, not the ~300 modeled). At 8 PSUM banks the
only remaining profitable shape is the bufs=3-equivalent 3-head ring.
"""
import sys

sys.path.insert(0, "/opt/trn_rl_repo")

import numpy as np

import concourse.bacc as bacc
import concourse.tile as tile
from concourse import mybir
from concourse.bass_utils import run_bass_kernel_spmd

F32 = mybir.dt.float32
F32R = mybir.dt.float32r
BF16 = mybir.dt.bfloat16

B, SQ, SE = 16, 1024, 1024
C, DK, DV, H = 256, 64, 64, 8
BN_EPS = 1e-5
NEG_SLOPE = 0.01
N_CORES = 8
BL = B // N_CORES
P = 128
NT = SE // P   # 8 key tiles
NQT = SQ // P  # 8 query tiles
SCH = 2        # score sc-chunks per sp tile
SCW = SQ // SCH  # 512
NSLOTS = BL * H


def build_kernel(n_cores=N_CORES, with_collective=True):
    nc = bacc.Bacc("TRN2", target_bir_lowering=False, debug=False,
                   num_devices=n_cores)

    qt_d = nc.declare_dram_parameter("qt", [BL, 2, P, SQ], BF16, isOutput=False)
    xt_d = nc.declare_dram_parameter("xt", [BL, 2, P, SE], BF16, isOutput=False)
    wqk0_d = nc.declare_dram_parameter("wqk0", [P, 2, 2, 2, DK], BF16,
                                       isOutput=False)
    wqkr_d = nc.declare_dram_parameter("wqkr", [P, 2, 2, H - 2, DK], BF16,
                                       isOutput=False)
    wv_d = nc.declare_dram_parameter("wv", [P, 2, H * DV], BF16, isOutput=False)
    wp_d = nc.declare_dram_parameter("wp", [P, H // 2, C], BF16, isOutput=False)
    gb_d = nc.declare_dram_parameter("gb", [P, 2, 2], F32, isOutput=False)
    id_d = nc.declare_dram_parameter("id128", [P, P], BF16, isOutput=False)
    y_d = nc.declare_dram_parameter("y", [BL, 2, P, SQ], BF16, isOutput=True)

    with tile.TileContext(nc) as tc:
        with (
            tc.tile_pool(name="const", bufs=1) as const,
            tc.tile_pool(name="qx", bufs=4) as qxp,       # qT/xT inputs
            tc.tile_pool(name="qk", bufs=4) as qkp,       # qhT/khT projections
            tc.tile_pool(name="vh", bufs=2) as vhp,       # vh_aug values
            tc.tile_pool(name="pt", bufs=2) as ptp,       # exp(scores)
            tc.tile_pool(name="o2", bufs=3) as o2p,       # normalized [q, e]
            tc.tile_pool(name="ot", bufs=2) as otp,       # oT [he, q]
            tc.tile_pool(name="pp", bufs=1) as ppp,       # projected p (both b)
            tc.tile_pool(name="sm", bufs=3) as sm,        # small scratch
            tc.tile_pool(name="yy", bufs=1) as yyp,       # y staging
            tc.tile_pool(name="fin", bufs=1) as fin,
            tc.tile_pool(name="sp_ps", bufs=3, space="PSUM") as sp_ps,  # 3x2bk
            tc.tile_pool(name="mm_ps", bufs=2, space="PSUM") as mm_ps,  # 2x1bk
            tc.tile_pool(name="dram", bufs=1, space="DRAM") as dram,
        ):
            wqk0_sb = const.tile([P, 2, 2, 2, DK], BF16, tag="wqk0")
            wqkr_sb = const.tile([P, 2, 2, H - 2, DK], BF16, tag="wqkr")

            def wslab(qk, m):
                """[P, 2(k), 2(heads), DK] weight slab for head-pair m."""
                if m == 0:
                    return wqk0_sb[:, qk]
                return wqkr_sb[:, qk, :, 2 * (m - 1):2 * m, :]
            wv_sb = const.tile([P, 2, H * DV], BF16, tag="wv")
            wp_sb = const.tile([P, H // 2, C], BF16, tag="wp")
            gb_sb = const.tile([P, 2, 2], F32, tag="gb")
            ident = const.tile([P, P], BF16, tag="ident")

            # ---------------- helpers ----------------
            p_sb = ppp.tile([P, 2, BL, SQ], BF16, tag="p")
            psq_scratch = sm.tile([P, SCW], BF16, tag="psq")
            sq_parts = fin.tile([P, 4 * BL], F32, tag="sqp")
            s_parts = fin.tile([P, 4 * BL], F32, tag="sp_")

            preps = {}
            vh_done = {}
            m_done = {}
            pts = {}
            o2s = {}
            avs = {}
            oTs = {}

            def proj_slab(wm, src, dst, m, sc0=0, sc1=SCH, act_evac=(),
                          pool=None):
                for sc in range(sc0, sc1):
                    pj = (pool or mm_ps).tile([P, SCW], F32,
                                              tag="sp" if pool else "mm")
                    for k in range(2):
                        nc.tensor.matmul(
                            pj[:],
                            wm[:, k, :, :],
                            src[:, k, sc * SCW:(sc + 1) * SCW],
                            start=(k == 0), stop=(k == 1))
                    dst_sl = dst[:, m, sc * SCW:(sc + 1) * SCW]
                    if sc in act_evac:
                        nc.scalar.activation(
                            out=dst_sl, in_=pj[:],
                            func=mybir.ActivationFunctionType.Copy)
                    else:
                        nc.vector.tensor_copy(dst_sl, pj[:])

            def prep_start(b):
                """Load qT/xT; project head-pair 0 (unblocks scores of h0)."""
                qT = qxp.tile([P, 2, SQ], BF16, tag="qx")
                xT = qxp.tile([P, 2, SE], BF16, tag="qx")
                if b == 0:
                    # q first (its projection runs while x still streams);
                    # split the last x chunk so the kh projection can start
                    # on the first half of the keys sooner
                    nc.sync.dma_start(out=qT[:, 0, :], in_=qt_d[b, 0])
                    nc.sync.dma_start(out=qT[:, 1, :], in_=qt_d[b, 1])
                    nc.sync.dma_start(out=xT[:, 0, :], in_=xt_d[b, 0])
                    nc.sync.dma_start(out=xT[:, 1, 0:SCW],
                                      in_=xt_d[b, 1][:, 0:SCW])
                    nc.sync.dma_start(out=xT[:, 1, SCW:SE],
                                      in_=xt_d[b, 1][:, SCW:SE])
                else:
                    for k in range(2):
                        nc.sync.dma_start(out=qT[:, k, :], in_=qt_d[b, k])
                    for k in range(2):
                        nc.sync.dma_start(out=xT[:, k, :], in_=xt_d[b, k])
                qhT = qkp.tile([P, H // 2, SQ], F32R, tag="qk")
                khT = qkp.tile([P, H // 2, SE], F32R, tag="qk")
                preps[b] = (qT, xT, qhT, khT, None)
                m_done[b] = -1
                # warmup: spread the four gating evacuations over DVE + ACT
                # and keep the k-slab PSUM out of the mm pool so it does not
                # wait on the q evacuations
                proj_slab(wslab(0, 0), qT, qhT, 0)
                proj_slab(wslab(1, 0), xT, khT, 0,
                          act_evac=(0,) if b == 0 else (),
                          pool=sp_ps if b == 0 else None)
                m_done[b] = 0

            def vh_alloc(b):
                qT, xT, qhT, khT, _ = preps[b]
                vh_aug = vhp.tile([P, NT, H, DV + 1], BF16, tag="vh")
                nc.vector.memset(vh_aug[:, :, :, DV:DV + 1], 1.0)
                preps[b] = (qT, xT, qhT, khT, vh_aug)

            def vproj(b, t0, t1):
                qT, xT, qhT, khT, vh_aug = preps[b]
                for t in range(t0, t1):
                    pj = mm_ps.tile([P, H * DV], F32, tag="mm")
                    for k in range(2):
                        nc.tensor.matmul(
                            pj[:], xT[:, k, t * P:(t + 1) * P], wv_sb[:, k, :],
                            start=(k == 0), stop=(k == 1))
                    nc.vector.tensor_copy(
                        vh_aug[:, t, :, 0:DV],
                        pj.rearrange("p (h e) -> p h e", h=H))

            def prep_units(b, with_start):
                units = []
                def qkslab(m):
                    proj_slab(wslab(1, m), preps[b][1], preps[b][3], m)
                    proj_slab(wslab(0, m), preps[b][0], preps[b][2], m)
                    m_done[b] = m

                if with_start:
                    units.append(lambda: prep_start(b))
                    units.append(lambda: (vh_alloc(b), vproj(b, 0, 4)))
                    units.append(lambda: (vproj(b, 4, 8),
                                          vh_done.__setitem__(b, True)))
                for m in range(1, H // 2):
                    units.append(lambda m=m: qkslab(m))
                return units

            pre_q = []
            post_q = []
            avAs = []

            def pop_units(q, n):
                for _ in range(min(n, len(q))):
                    q.pop(0)()

            def score_tiles(s, t0, t1, out):
                """Score tiles for head-slot s into sp tiles (PSUM)."""
                b, h = divmod(s, H)
                j, par = h // 2, 64 * (h % 2)
                while m_done.get(b, -1) < j and pre_q:
                    pop_units(pre_q, 1)
                qhT, khT = preps[b][2], preps[b][3]
                for t in range(t0, t1):
                    spt = sp_ps.tile([P, SCH, SCW], F32, tag="sp")
                    for sc in range(SCH):
                        nc.tensor.matmul(
                            spt[:, sc, :],
                            khT[par:par + 64, j, t * P:(t + 1) * P],
                            qhT[par:par + 64, j, sc * SCW:(sc + 1) * SCW],
                            start=True, stop=True)
                    out.append(spt)
                return out

            def emit_exp(s, sp_list):
                pt = ptp.tile([P, NT, SQ], BF16, tag="pt")
                pts[s] = pt
                for t in range(NT):
                    nc.scalar.activation(
                        out=pt[:, t, :],
                        in_=sp_list[t].rearrange("p a b -> p (a b)"),
                        func=mybir.ActivationFunctionType.Exp,
                        scale=1.0 / np.sqrt(DK).item())

            def av_and_norm(s):
                """AV ([q, e] layout, pt stationary) + per-partition norm."""
                b, h = divmod(s, H)
                if b > 0:
                    while b not in vh_done and pre_q:
                        pop_units(pre_q, 1)
                vh_aug = preps[b][4]
                pt = pts.pop(s)
                # normalization: r varies along partitions (q)
                r = sm.tile([P, NQT], F32, tag="r")
                o2 = o2p.tile([P, NQT, DV], BF16, tag="o2")
                o2s[s] = o2
                for half in range(2):
                    avt = mm_ps.tile([P, 4, DV + 1], F32, tag="mm")
                    for qi in range(4):
                        qt = half * 4 + qi
                        for kt in range(NT):
                            nc.tensor.matmul(
                                avt[:, qi, :],
                                pt[:, kt, qt * P:(qt + 1) * P],
                                vh_aug[:, kt, h, :],
                                start=(kt == 0), stop=(kt == NT - 1))
                    nc.vector.reciprocal(
                        r[:, half * 4:(half + 1) * 4], avt[:, :, DV])
                    for qi in range(4):
                        qt = half * 4 + qi
                        nc.vector.tensor_scalar(
                            o2[:, qt, :], avt[:, qi, 0:DV],
                            r[:, qt:qt + 1], None, mybir.AluOpType.mult)

            def transp_pair(s_even, split_evac=False):
                """Transpose heads (s_even, s_even+1) into oT[:, j, :]."""
                b, h = divmod(s_even, H)
                j = h // 2
                if j == 0:
                    oTs[b] = otp.tile([P, H // 2, SQ], BF16, tag="ot",
                                      name=f"oT{b}")
                oT = oTs[b]
                tp = mm_ps.tile([P, NQT, P], BF16, tag="mm")
                o2a = o2s.pop(s_even)
                o2b = o2s.pop(s_even + 1)
                tpf = tp.rearrange("p a b -> p (a b)")
                for half in range(2):
                    for qt in range(4 * half, 4 * half + 4):
                        nc.tensor.transpose(tp[0:64, qt, :], o2a[:, qt, :],
                                            ident)
                        nc.tensor.transpose(tp[64:P, qt, :], o2b[:, qt, :],
                                            ident)
                    if split_evac:
                        nc.vector.tensor_copy(
                            oT[:, j, half * SCW:(half + 1) * SCW],
                            tpf[:, half * SCW:(half + 1) * SCW])
                if not split_evac:
                    nc.vector.tensor_copy(oT[:, j, :], tpf)

            def out_proj_sc(b, ct, sc, g0=0, g1=H // 2, psq_eng=None):
                """p[c, s] for head-pair groups [g0, g1) of one (ct, sc)
                chunk; finishes BN partial stats when g1 == H//2."""
                oT = oTs[b]
                pj = mm_ps.tile([P, SCW], F32, tag="mm")
                for g in range(g0, g1):
                    nc.tensor.matmul(
                        pj[:],
                        wp_sb[:, g, ct * P:(ct + 1) * P],
                        oT[:, g, sc * SCW:(sc + 1) * SCW],
                        start=(g == g0), stop=(g == g1 - 1))
                col = 4 * ct + 2 * b + sc
                psl = p_sb[:, ct, b, sc * SCW:(sc + 1) * SCW]
                if g1 < H // 2:
                    # partial: stash in p_sb, no stats yet
                    nc.vector.tensor_scalar(
                        psl, pj[:], 1.0, 0.0,
                        mybir.AluOpType.mult, mybir.AluOpType.add)
                    return
                if g0 > 0:
                    # combine with the stashed partial + finish stats
                    nc.vector.scalar_tensor_tensor(
                        psl, pj[:], 1.0, psl,
                        mybir.AluOpType.mult, mybir.AluOpType.add,
                        accum_out=s_parts[:, col:col + 1])
                else:
                    nc.vector.tensor_scalar(
                        psl, pj[:],
                        1.0, 0.0, mybir.AluOpType.mult, mybir.AluOpType.add,
                        accum_out=s_parts[:, col:col + 1])
                if psq_eng == "act":
                    # tail: ACT is idle there and Square is in every table
                    nc.scalar.activation(
                        out=psq_scratch[:], in_=psl,
                        func=mybir.ActivationFunctionType.Square,
                        accum_out=sq_parts[:, col:col + 1])
                else:
                    nc.vector.scalar_tensor_tensor(
                        psq_scratch[:], psl, 1.0, psl,
                        mybir.AluOpType.mult, mybir.AluOpType.mult,
                        accum_out=sq_parts[:, col:col + 1])

            # ---------------- emission ----------------
            # input DMAs + first projections
            nc.sync.dma_start(out=wqk0_sb, in_=wqk0_d[:])

            # PE p-state ramp during the DMA wait
            warm = const.tile([64, SCW], BF16, tag="warm")
            nc.vector.memset(warm, 0.0)
            for i in range(4):
                wt = sp_ps.tile([P, SCH, SCW], F32, tag="sp")
                nc.tensor.matmul(wt[0:64, 0, :], warm[:, 0:64], warm[:],
                                 start=True, stop=True)

            prep_start(0)
            nc.sync.dma_start(out=wqkr_sb, in_=wqkr_d[:])
            nc.sync.dma_start(out=wv_sb, in_=wv_d[:])
            nc.sync.dma_start(out=ident, in_=id_d[:])
            nc.sync.dma_start(out=wp_sb, in_=wp_d[:])
            nc.sync.dma_start(out=gb_sb, in_=gb_d[:])
            sp_cur = score_tiles(0, 0, NT, [])
            vh_alloc(0)
            vproj(0, 0, 8)
            vh_done[0] = True
            pre_q.extend(prep_units(0, with_start=False))
            pre_q.extend(prep_units(1, with_start=True))

            for s in range(NSLOTS):
                emit_exp(s, sp_cur)
                if s >= 1:
                    av_and_norm(s - 1)
                if s == NSLOTS - 1:
                    # head 14's rows of oT j3 transpose+evacuate during the
                    # last head's exps -- only head 15 remains on the tail
                    o2a14 = o2s[s - 1]
                    oTL = oTs[BL - 1]
                    tpa = mm_ps.tile([P, NQT, P], BF16, tag="mm", name="tpa")
                    for qt in range(NQT):
                        nc.tensor.transpose(tpa[0:64, qt, :],
                                            o2a14[:, qt, :], ident)
                    nc.vector.tensor_copy(
                        oTL[0:64, H // 2 - 1, :],
                        tpa[0:64].rearrange("p a b -> p (a b)"))
                    # last head's AV over the first half of the keys runs
                    # as soon as those exps land; evacuated to SBUF so the
                    # tail only accumulates keys 512:1024 and combines
                    ptL15 = pts[s]
                    vhL = preps[BL - 1][4]
                    for half in range(2):
                        avt = mm_ps.tile([P, 4, DV + 1], F32, tag="mm",
                                         name=f"avA{half}")
                        for qi in range(4):
                            qt = half * 4 + qi
                            for kt in range(NT // 2):
                                nc.tensor.matmul(
                                    avt[:, qi, :],
                                    ptL15[:, kt, qt * P:(qt + 1) * P],
                                    vhL[:, kt, H - 1, :],
                                    start=(kt == 0), stop=(kt == NT // 2 - 1))
                        avA = sm.tile([P, 4, DV + 1], F32, tag="avp",
                                      name=f"avAs{half}")
                        avAs.append(avA)
                        nc.vector.tensor_copy(avA, avt)
                if s >= 2 and s % 2 == 0:
                    transp_pair(s - 2)
                    if (s - 2) % H == 6:  # j3 done -> queue b's out_proj
                        bb = (s - 2) // H
                        for sc in range(SCH):
                            for ct in range(2):
                                post_q.append(
                                    lambda ct=ct, sc=sc, bb=bb: out_proj_sc(
                                        bb, ct, sc))
                    if s == NSLOTS - 2:
                        # partial out-proj (head-pairs g0-g2) for the last
                        # batch -- oT j0..j2 are evacuated by now
                        for sc in range(SCH):
                            for ct in range(2):
                                post_q.append(
                                    lambda ct=ct, sc=sc: out_proj_sc(
                                        BL - 1, ct, sc, g0=0, g1=3))
                if s + 1 < NSLOTS:
                    sp_cur = score_tiles(s + 1, 0, 2, [])
                pop_units(pre_q, 2 if s < 2 else 1)
                if s + 1 < NSLOTS:
                    score_tiles(s + 1, 2, 4, sp_cur)
                pop_units(post_q, 2 if s >= NSLOTS - 2 else 1)
                if s + 1 < NSLOTS:
                    score_tiles(s + 1, 4, NT, sp_cur)

            # ---- BN finalize/apply helpers ----
            n_total = float(B * SQ) if with_collective else float(BL * SQ)
            stats = fin.tile([P, 2, 2], F32, tag="stats")  # [c, ct, {s, s2}]
            a_ap = fin.tile([P, 2], F32, tag="a")
            b_ap = fin.tile([P, 2], F32, tag="b")
            mean2 = fin.tile([P, 2], F32, tag="mean2")
            msq2 = fin.tile([P, 2], F32, tag="msq2")
            var2 = fin.tile([P, 2], F32, tag="var2")
            iv2 = fin.tile([P, 2], F32, tag="iv2")
            rstd2 = fin.tile([P, 2], F32, tag="rstd2")
            bm2 = fin.tile([P, 2], F32, tag="bm2")
            y_all = yyp.tile([P, 2, BL, SQ], BF16, tag="yall")

            def stats_ct(ct):
                nc.vector.tensor_reduce(
                    stats[:, ct:ct + 1, 0],
                    s_parts[:, 4 * ct:4 * ct + 4].rearrange(
                        "p (c x) -> p c x", c=1),
                    mybir.AxisListType.X, mybir.AluOpType.add)
                nc.vector.tensor_reduce(
                    stats[:, ct:ct + 1, 1],
                    sq_parts[:, 4 * ct:4 * ct + 4].rearrange(
                        "p (c x) -> p c x", c=1),
                    mybir.AxisListType.X, mybir.AluOpType.add)

            def finalize_ct(ct, g_sb):
                c = slice(ct, ct + 1)
                nc.vector.tensor_scalar(mean2[:, c], g_sb[:, c, 0],
                                        1.0 / n_total, None,
                                        mybir.AluOpType.mult)
                nc.vector.tensor_scalar(msq2[:, c], g_sb[:, c, 1],
                                        1.0 / n_total, None,
                                        mybir.AluOpType.mult)
                nc.vector.tensor_mul(var2[:, c], mean2[:, c], mean2[:, c])
                nc.vector.tensor_sub(var2[:, c], msq2[:, c], var2[:, c])
                nc.vector.tensor_scalar(var2[:, c], var2[:, c], 1.0, BN_EPS,
                                        mybir.AluOpType.mult,
                                        mybir.AluOpType.add)
                nc.vector.reciprocal(iv2[:, c], var2[:, c])
                # rstd = sqrt(1/(var+eps)); Sqrt set also holds Prelu
                nc.scalar.activation(out=rstd2[:, c], in_=iv2[:, c],
                                     func=mybir.ActivationFunctionType.Sqrt)
                nc.vector.tensor_mul(a_ap[:, c], rstd2[:, c], gb_sb[:, c, 0])
                nc.vector.tensor_mul(bm2[:, c], mean2[:, c], a_ap[:, c])
                nc.vector.tensor_sub(b_ap[:, c], gb_sb[:, c, 1], bm2[:, c])

            def apply_act(b, ct):
                nc.scalar.activation(
                    out=y_all[:, ct, b, :], in_=p_sb[:, ct, b, :],
                    func=mybir.ActivationFunctionType.Prelu,
                    scale=a_ap[:, ct:ct + 1], bias=b_ap[:, ct:ct + 1],
                    alpha=NEG_SLOPE)

            def apply_vec(b, ct):
                yt = sm.tile([P, SQ], BF16, tag="yt")
                nc.vector.tensor_scalar(
                    yt, p_sb[:, ct, b, :], a_ap[:, ct:ct + 1],
                    b_ap[:, ct:ct + 1],
                    mybir.AluOpType.mult, mybir.AluOpType.add)
                nc.vector.scalar_tensor_tensor(
                    y_all[:, ct, b, :], yt, NEG_SLOPE, yt,
                    mybir.AluOpType.mult, mybir.AluOpType.max)

            def finish_ct(ct):
                finalize_ct(ct, stats)
                apply_act(0, ct)
                if ct == 0:
                    apply_act(1, ct)
                else:
                    apply_vec(1, ct)
                nc.sync.dma_start(out=y_d[0, ct], in_=y_all[:, ct, 0, :])
                nc.sync.dma_start(out=y_d[1, ct], in_=y_all[:, ct, 1, :])

            # ---------------- attention tail ----------------
            # last head: av / norm / transpose / evacuate / g3 out-proj,
            # pipelined per qt-half so every engine starts early
            pop_units(post_q, len(post_q))
            # hoist the sqrt-table load off the critical path: everything
            # ACT does from here on (Square/Sqrt/Prelu) lives in the
            # sqrt_and_others set, so switch tables now while ACT is idle.
            # The dummy writes into y_all (which has real readers) so it
            # survives dead-code elimination; the apply overwrites it.
            nc.scalar.activation(out=y_all[0:1, 0, 0, 0:1],
                                 in_=ident[0:1, 0:1],
                                 func=mybir.ActivationFunctionType.Sqrt)
            sL = NSLOTS - 1
            bL = BL - 1
            vh_aug = preps[bL][4]
            ptL = pts.pop(sL)
            o2a = o2s.pop(sL - 1)
            o2b = o2p.tile([P, NQT, DV], BF16, tag="o2", name="o2last")
            rL = sm.tile([P, NQT], F32, tag="r", name="rlast")
            oT = oTs[bL]
            for half in range(2):
                avt = mm_ps.tile([P, 4, DV + 1], F32, tag="mm",
                                 name=f"avl{half}")
                for qi in range(4):
                    qt = half * 4 + qi
                    for kt in range(NT // 2, NT):
                        nc.tensor.matmul(
                            avt[:, qi, :],
                            ptL[:, kt, qt * P:(qt + 1) * P],
                            vh_aug[:, kt, H - 1, :],
                            start=(kt == NT // 2), stop=(kt == NT - 1))
                cmb = sm.tile([P, 4, DV + 1], F32, tag="avp",
                              name=f"cmb{half}")
                nc.vector.scalar_tensor_tensor(
                    cmb, avt[:], 1.0, avAs[half],
                    mybir.AluOpType.mult, mybir.AluOpType.add)
                nc.vector.reciprocal(
                    rL[:, half * 4:(half + 1) * 4], cmb[:, :, DV])
                for qi in range(4):
                    qt = half * 4 + qi
                    if qi < 2:
                        nc.vector.tensor_scalar(
                            o2b[:, qt, :], cmb[:, qi, 0:DV],
                            rL[:, qt:qt + 1], None, mybir.AluOpType.mult)
                    else:
                        nc.scalar.activation(
                            out=o2b[:, qt, :], in_=cmb[:, qi, 0:DV],
                            func=mybir.ActivationFunctionType.Copy,
                            scale=rL[:, qt:qt + 1])
            for half in range(2):
                tph = mm_ps.tile([P, 4, P], BF16, tag="mm",
                                 name=f"tpl{half}")
                for qi in range(4):
                    qt = half * 4 + qi
                    nc.tensor.transpose(tph[64:P, qi, :], o2b[:, qt, :],
                                        ident)
                nc.vector.tensor_copy(
                    oT[64:P, H // 2 - 1, half * SCW:(half + 1) * SCW],
                    tph[64:P].rearrange("p a b -> p (a b)"))
            # final chunks ct-major; finalize both cts before the applies
            out_proj_sc(bL, 0, 0, g0=3, psq_eng="act")
            out_proj_sc(bL, 0, 1, g0=3, psq_eng="act")
            stats_ct(0)
            out_proj_sc(bL, 1, 0, g0=3, psq_eng="act")
            out_proj_sc(bL, 1, 1, g0=3)
            stats_ct(1)
            if not with_collective:
                finalize_ct(0, stats)
                finalize_ct(1, stats)
                apply_act(0, 0)
                apply_vec(1, 0)
                nc.sync.dma_start(out=y_d[0, 0], in_=y_all[:, 0, 0, :])
                nc.sync.dma_start(out=y_d[1, 0], in_=y_all[:, 0, 1, :])
                apply_act(0, 1)
                apply_vec(1, 1)
                nc.sync.dma_start(out=y_d[0, 1], in_=y_all[:, 1, 0, :])
                nc.sync.dma_start(out=y_d[1, 1], in_=y_all[:, 1, 1, :])

            # ---- collective path: all-reduce stats, then finalize ----
            if with_collective:
                ar_in = dram.tile([P, 4], F32)
                ar_out = dram.tile([P, 4], F32)
                nc.sync.dma_start(out=ar_in[:],
                                  in_=stats.rearrange("p a b -> p (a b)"))
                nc.gpsimd.collective_compute(
                    "AllReduce", mybir.AluOpType.add,
                    replica_groups=[list(range(n_cores))],
                    ins=[ar_in.opt()], outs=[ar_out.opt()])
                g_sb = fin.tile([P, 2, 2], F32, tag="g")
                nc.sync.dma_start(out=g_sb.rearrange("p a b -> p (a b)"),
                                  in_=ar_out[:])
                for ct in range(2):
                    finalize_ct(ct, g_sb)
                    apply_act(0, ct)
                    apply_vec(1, ct)
                    nc.sync.dma_start(out=y_d[0, ct],
                                      in_=y_all[:, ct, 0, :])
                    nc.sync.dma_start(out=y_d[1, ct],
                                      in_=y_all[:, ct, 1, :])

    nc.compile()
    return nc


def prep_weights(Wq, Wk, Wv, Wp, gamma, beta):
    import ml_dtypes
    wq = np.ascontiguousarray(
        Wq.transpose(2, 0, 1).reshape(2, P, H, DK)
        .transpose(1, 0, 2, 3)).astype(ml_dtypes.bfloat16)
    wk = np.ascontiguousarray(
        Wk.transpose(2, 0, 1).reshape(2, P, H, DK)
        .transpose(1, 0, 2, 3)).astype(ml_dtypes.bfloat16)
    wqk = np.stack([wq, wk], axis=1)  # [P, 2(qk), 2(kc), H, DK]
    wqk0 = np.ascontiguousarray(wqk[:, :, :, 0:2, :])
    wqkr = np.ascontiguousarray(wqk[:, :, :, 2:, :])
    wv = np.ascontiguousarray(
        Wv.transpose(2, 0, 1).reshape(2, P, H * DV)
        .transpose(1, 0, 2)).astype(ml_dtypes.bfloat16)
    # wp: [128 (he within group), group, c] with he = h*64+e head-major
    wpT = Wp.T.reshape(H // 2, P, C)  # [g, he%128, c]
    wp = np.ascontiguousarray(wpT.transpose(1, 0, 2)).astype(ml_dtypes.bfloat16)
    # gamma/beta in [c%128, ct, {gamma,beta}]
    gb = np.stack([gamma.reshape(2, P), beta.reshape(2, P)], axis=-1)
    gb = np.ascontiguousarray(gb.transpose(1, 0, 2)).astype(np.float32)
    ident = np.eye(P, dtype=ml_dtypes.bfloat16)
    return (wqk0, wqkr), wv, wp, gb, ident


_NC_CACHE = {}


def kernel(x, q, Wq, Wk, Wv, Wp, gamma, beta):
    x = np.asarray(x, dtype=np.float32)
    q = np.asarray(q, dtype=np.float32)
    (wqk0, wqkr), wv, wp, gb, ident = prep_weights(
        np.asarray(Wq, np.float32), np.asarray(Wk, np.float32),
        np.asarray(Wv, np.float32), np.asarray(Wp, np.float32),
        np.asarray(gamma, np.float32), np.asarray(beta, np.float32))

    if "nc" not in _NC_CACHE:
        _NC_CACHE["nc"] = build_kernel()
    nc = _NC_CACHE["nc"]

    import ml_dtypes

    # host-side transpose: [BL, S, C] -> [BL, 2, 128, S] (bf16)
    def t_in(a):
        return np.ascontiguousarray(
            a.transpose(0, 2, 1).reshape(a.shape[0], 2, P, a.shape[1])
        ).astype(ml_dtypes.bfloat16)

    in_maps = []
    for i in range(N_CORES):
        in_maps.append({
            "qt": t_in(q[i * BL:(i + 1) * BL]),
            "xt": t_in(x[i * BL:(i + 1) * BL]),
            "wqk0": wqk0, "wqkr": wqkr, "wv": wv, "wp": wp, "gb": gb,
            "id128": ident,
        })
    res = run_bass_kernel_spmd(nc, in_maps, list(range(N_CORES)))
    outs = []
    for i in range(N_CORES):
        y = np.asarray(res.results[i]["y"]).astype(np.float32)
        y = y.reshape(BL, 2, P, SQ).transpose(0, 3, 1, 2).reshape(BL, SQ, C)
        outs.append(y)
    return np.concatenate(outs, axis=0)
